# revision 17
# baseline (speedup 1.0000x reference)
"""GQA attention kernel for Trainium2, 8 NeuronCores.

Sharding: data-parallel over batch (B=2) x tensor-parallel over KV heads
(HKV=4) -> 8 cores.  Core c handles batch b=c//4, kv-head j=c%4 with its
G=4 query heads.  out_proj is row-parallel; partials are reduced on host.

Layout strategy (v2):
  - Projections in NATURAL orientation (out[seq, feat]): lhsT = hiddenT
    chunk, rhs = W chunk.  RoPE and rmsnorm then operate along the free
    dim (cheap DVE/Pool ops, no partition reductions).
  - rsqrt for rmsnorm is exp(-0.5*ln(x)) on ACT: both funcs live in the
    natural_log_exp_and_others activation table together with the softmax
    Exp, so the ACT engine never reloads its table.
  - qT / kT for the scores matmuls are produced by DMA-transpose
    (crossbar) instructions; kT's row-64..127 duplicate is folded into
    the same transpose by duplicating kn columns beforehand.
  - scoresT[key, q] = kT^T @ qT per head, exp on ACT (the hard floor:
    ~110us of exp at 0.833 ns/elem), probabilities pT kept in SBUF for a
    full 512-q block.
  - PV in flipped orientation: out[q, d+1] with lhsT = pT chunk,
    rhs = v (with ones column -> denominator lands as column 64).  N=65
    per matmul instead of 512 -> half the PE rows of the baseline.
    Normalization is a per-partition reciprocal + broadcast multiply.
  - oT via DMA-transpose feeds a row-parallel out_proj; partials DMA'd
    per 128-row chunk.
PSUM budget (8 banks): pq 1 | scoresA 2 | scoresB 2 | oraw 2 | y 1.
The lead-in k/v/q chains round-robin across all five slots.
"""

import numpy as np
import ml_dtypes

import concourse.bacc as bacc
import concourse.mybir as mybir
from concourse.tile import TileContext

BF16 = mybir.dt.bfloat16
F32 = mybir.dt.float32
AL = mybir.AluOpType
AF = mybir.ActivationFunctionType
AX = mybir.AxisListType

B, S, HID = 2, 2048, 1024
H, HKV, D = 16, 4, 64
G = H // HKV          # 4 query heads per kv head
QSEL = 2 * G * D      # 512: own 256 cols + rope-partner 256 cols
ROPE_BASE = 10000.0
EPS = float(np.finfo(np.float32).eps)
NSC = S // 128        # 16 seq chunks
NIC = 4               # 512-wide q blocks

NB = ml_dtypes.bfloat16

_cache: dict = {}


def _build(use_mask: bool, use_bias: bool):
    nc = bacc.Bacc("TRN2", target_bir_lowering=False)

    hT = nc.dram_tensor("hT", [8, 128, S], BF16, kind="ExternalInput")
    wq = nc.dram_tensor("wq", [8, 128, QSEL], BF16, kind="ExternalInput")
    wk = nc.dram_tensor("wk", [8, 128, 128], BF16, kind="ExternalInput")
    wv = nc.dram_tensor("wv", [8, 128, 64], BF16, kind="ExternalInput")
    wo = nc.dram_tensor("wo", [2, 128, HID], BF16, kind="ExternalInput")
    csq = nc.dram_tensor("csq", [NSC, 128, 2, 256], BF16, kind="ExternalInput")
    csk = nc.dram_tensor("csk", [NSC, 128, 2, 64], BF16, kind="ExternalInput")
    y = nc.dram_tensor("y", [NSC, 128, HID], F32, kind="ExternalOutput")
    mk = (
        nc.dram_tensor("mk", [NSC, 128, S], F32, kind="ExternalInput")
        if use_mask
        else None
    )
    if use_bias:
        brq = nc.dram_tensor("brq", [1, QSEL], BF16, kind="ExternalInput")
        brk = nc.dram_tensor("brk", [1, 128], BF16, kind="ExternalInput")
        brv = nc.dram_tensor("brv", [1, 64], BF16, kind="ExternalInput")

    with TileContext(nc) as tc:
        with (
            tc.tile_pool(name="const", bufs=1) as cp,
            tc.tile_pool(name="proj", bufs=1) as pj,
            tc.tile_pool(name="rt", bufs=3) as rt,
            tc.tile_pool(name="ro", bufs=6) as rop,
            tc.tile_pool(name="stat", bufs=3) as stp,
            tc.tile_pool(name="pT", bufs=20) as ptp,
            tc.tile_pool(name="onat", bufs=3) as onp_,
            tc.tile_pool(name="oTp", bufs=3) as otp,
            tc.tile_pool(name="ysb", bufs=2) as yp,
            tc.tile_pool(name="maskp", bufs=3) as mp,
            tc.tile_pool(name="ps", bufs=1, space="PSUM") as ps,
        ):
            # ---- persistent tiles ------------------------------------
            wo_sb = cp.tile([128, 2, HID], BF16)
            for cc in range(2):
                nc.sync.dma_start(out=wo_sb[:, cc, :], in_=wo[cc])
            v_sb = cp.tile([128, NSC, 66], BF16)
            nc.vector.memset(v_sb[:, :, 64:65], 1.0)
            eps_sb = cp.tile([128, 1], F32)
            nc.vector.memset(eps_sb[:], EPS)
            qT = cp.tile([128, 2, S], BF16)
            kT = cp.tile([128, S], BF16)

            # ---- projection-phase constants --------------------------
            # small weights first so the first k/v matmuls start early
            wk_sb = pj.tile([128, 8, 128], BF16)
            nc.sync.dma_start(out=wk_sb[:], in_=wk[:].rearrange("a b c -> b a c"))
            wv_sb = pj.tile([128, 8, 64], BF16)
            nc.sync.dma_start(out=wv_sb[:], in_=wv[:].rearrange("a b c -> b a c"))
            csk_sb = pj.tile([128, NSC, 2, 64], BF16)
            nc.sync.dma_start(out=csk_sb[:], in_=csk[:].rearrange("a b c d -> b a c d"))
            hT_sb = pj.tile([128, 8, S], BF16)
            for ko in range(8):
                nc.sync.dma_start(out=hT_sb[:, ko, :], in_=hT[ko])
            wq_sb = pj.tile([128, 8, QSEL], BF16)
            nc.sync.dma_start(out=wq_sb[:], in_=wq[:].rearrange("a b c -> b a c"))
            csq_sb = pj.tile([128, NSC, 2, 256], BF16)
            nc.sync.dma_start(out=csq_sb[:], in_=csq[:].rearrange("a b c d -> b a c d"))
            if use_bias:
                ones1 = cp.tile([1, 128], BF16)
                nc.vector.memset(ones1[:], 1.0)
                brq_sb = cp.tile([1, QSEL], BF16)
                nc.sync.dma_start(out=brq_sb[:], in_=brq[:])
                brk_sb = cp.tile([1, 128], BF16)
                nc.sync.dma_start(out=brk_sb[:], in_=brk[:])
                brv_sb = cp.tile([1, 64], BF16)
                nc.sync.dma_start(out=brv_sb[:], in_=brv[:])

            # PSUM slot round-robin for the lead-in projection chains
            SLOTS = ["scA", "scB", "oraw", "py", "pq"]
            slot_i = [0]

            def next_slot():
                s = SLOTS[slot_i[0] % len(SLOTS)]
                slot_i[0] += 1
                return s

            def rsqrt_batch(rm, tag):
                """rm: [128, n] f32 sums of squares/64 -> (rm+eps)^-0.5 via
                exp(-0.5 * ln(rm + eps)); stays in the exp table set."""
                n = rm.shape[1]
                ln_t = stp.tile([128, n], F32, tag=tag + "_ln")
                nc.scalar.activation(ln_t[:], rm[:], AF.Ln, bias=eps_sb[:])
                rc = stp.tile([128, n], F32, tag=tag + "_rc")
                nc.scalar.activation(rc[:], ln_t[:], AF.Exp, scale=-0.5)
                return rc

            def kv_sub(sc, rmk4, i, kros):
                ssl = slice(sc * 128, (sc + 1) * 128)
                pk = ps.tile([128, 2, 64], F32, tag=next_slot())
                pv = ps.tile([128, 64], F32, tag=next_slot())
                for ko in range(8):
                    st, sp = ko == 0, (ko == 7 and not use_bias)
                    nc.tensor.matmul(
                        pk[:], lhsT=hT_sb[:, ko, ssl], rhs=wk_sb[:, ko, :],
                        start=st, stop=sp,
                    )
                for ko in range(8):
                    st, sp = ko == 0, (ko == 7 and not use_bias)
                    nc.tensor.matmul(
                        pv[:], lhsT=hT_sb[:, ko, ssl], rhs=wv_sb[:, ko, :],
                        start=st, stop=sp,
                    )
                if use_bias:
                    nc.tensor.matmul(pk[:], lhsT=ones1[:], rhs=brk_sb[:],
                                     start=False, stop=True)
                    nc.tensor.matmul(pv[:], lhsT=ones1[:], rhs=brv_sb[:],
                                     start=False, stop=True)
                t12k = rt.tile([128, 2, 64], BF16, tag="t12k")
                nc.vector.tensor_tensor(t12k[:], pk[:], csk_sb[:, sc, :, :], AL.mult)
                kro = rop.tile([128, 64], BF16, tag="kro")
                nc.gpsimd.tensor_tensor(kro[:], t12k[:, 0, :], t12k[:, 1, :], AL.add)
                kros.append(kro)
                sqk = rt.tile([128, 64], BF16, tag="sqk")
                nc.gpsimd.scalar_tensor_tensor(
                    sqk[:], kro[:], 1.0 / 64.0, kro[:], AL.mult, AL.mult
                )
                nc.vector.tensor_reduce(rmk4[:, i:i + 1], sqk[:], AX.X, AL.add)
                nc.vector.tensor_copy(v_sb[:, sc, 0:64], pv[:])

            def kv_fin(g, rmk4, kros):
                rck = rsqrt_batch(rmk4, "rck")
                for i in range(4):
                    sc = 4 * g + i
                    kn2 = rt.tile([128, 2, 64], BF16, tag="kn2")
                    nc.vector.tensor_scalar_mul(kn2[:, 0, :], kros[i][:], rck[:, i:i + 1])
                    nc.gpsimd.tensor_copy(kn2[:, 1, :], kn2[:, 0, :])
                    nc.sync.dma_start_transpose(
                        out=kT[:, sc * 128:(sc + 1) * 128], in_=kn2[:]
                    )

            def q_sub(sc, rms16, i, qros, lead=False):
                ssl = slice(sc * 128, (sc + 1) * 128)
                pq = ps.tile([128, 2, 256], F32, tag=(next_slot() if lead else "pq"))
                for ko in range(8):
                    st, sp = ko == 0, (ko == 7 and not use_bias)
                    nc.tensor.matmul(
                        pq[:], lhsT=hT_sb[:, ko, ssl], rhs=wq_sb[:, ko, :],
                        start=st, stop=sp,
                    )
                if use_bias:
                    nc.tensor.matmul(pq[:], lhsT=ones1[:], rhs=brq_sb[:],
                                     start=False, stop=True)
                q_sub_tail(sc, rms16, i, qros, pq)

            def q_sub_mm(sc, pq_box, ko0, lead=False):
                """two accumulation matmuls of the q projection for chunk sc"""
                ssl = slice(sc * 128, (sc + 1) * 128)
                if ko0 == 0:
                    pq_box.append(
                        ps.tile([128, 2, 256], F32,
                                tag=(next_slot() if lead else "pq"), name="pq")
                    )
                pq = pq_box[0]
                for ko in (ko0, ko0 + 1):
                    st = ko == 0
                    sp = ko == 7 and not use_bias
                    nc.tensor.matmul(
                        pq[:], lhsT=hT_sb[:, ko, ssl], rhs=wq_sb[:, ko, :],
                        start=st, stop=sp,
                    )
                if ko0 == 6 and use_bias:
                    nc.tensor.matmul(pq[:], lhsT=ones1[:], rhs=brq_sb[:],
                                     start=False, stop=True)

            def q_sub_tail(sc, rms16, i, qros, pq):
                t12 = rt.tile([128, 2, 256], BF16, tag="t12")
                nc.vector.tensor_tensor(t12[:], pq[:], csq_sb[:, sc, :, :], AL.mult)
                qro = rop.tile([128, 4, 64], BF16, tag="qro")
                nc.gpsimd.tensor_tensor(
                    qro[:].rearrange("p h d -> p (h d)"), t12[:, 0, :], t12[:, 1, :],
                    AL.add,
                )
                qros.append(qro)
                sqq = rt.tile([128, 4, 64], BF16, tag="sqq")
                nc.gpsimd.scalar_tensor_tensor(
                    sqq[:], qro[:], 1.0 / 64.0, qro[:], AL.mult, AL.mult
                )
                nc.vector.tensor_reduce(rms16[:, 4 * i:4 * i + 4], sqq[:], AX.X, AL.add)

            def q_fin(ic, rms16, qros):
                rcq = rsqrt_batch(rms16, "rcq")
                for i in range(4):
                    sc = 4 * ic + i
                    qn = rt.tile([128, 4, 64], BF16, tag="qn")
                    nc.vector.tensor_tensor(
                        qn[:], qros[i][:],
                        rcq[:, 4 * i:4 * i + 4, None].to_broadcast((128, 4, 64)),
                        AL.mult,
                    )
                    nc.sync.dma_start_transpose(
                        out=qT[:, :, sc * 128:(sc + 1) * 128], in_=qn[:]
                    )

            def norm_half(h, oraw, box):
                # oraw: [128, 8, 128] psum, slices (s2, hd) at s2*4+hd, col 64 = denom
                rcp = stp.tile([128, 8], F32, tag="rcp")
                nc.vector.reciprocal(rcp[:], oraw[:, :, 64:65])
                for s2 in range(2):
                    onat = onp_.tile([128, 4, 64], BF16, tag="onat")
                    nc.vector.tensor_tensor(
                        onat[:], oraw[:, s2 * 4:(s2 + 1) * 4, 0:64],
                        rcp[:, s2 * 4:(s2 + 1) * 4, None].to_broadcast((128, 4, 64)),
                        AL.mult,
                    )
                    oTt = otp.tile([128, 2, 128], BF16, tag="oTt")
                    nc.sync.dma_start_transpose(out=oTt[:], in_=onat[:])
                    box.append(oTt)

            def outproj_sub(gc, oTt):
                ysb = yp.tile([128, HID], F32, tag="ysb")
                for ec in range(2):
                    py = ps.tile([128, 512], F32, tag="py")
                    for cc in range(2):
                        nc.tensor.matmul(
                            py[:], lhsT=oTt[:, cc, :],
                            rhs=wo_sb[:, cc, ec * 512:(ec + 1) * 512],
                            start=(cc == 0), stop=(cc == 1),
                        )
                    nc.vector.tensor_copy(ysb[:, ec * 512:(ec + 1) * 512], py[:])
                # y stores go out on the gpsimd swdge queue so they never
                # delay the latency-critical qT/oT transposes issued from SP
                nc.gpsimd.dma_start(out=y[gc], in_=ysb[:])

            # ---- lead-in: k/v for all chunks, then q for ic 0 --------
            for g in range(4):
                rmk4 = stp.tile([128, 4], F32, tag="rmk4")
                kros = []
                for i in range(4):
                    kv_sub(4 * g + i, rmk4, i, kros)
                kv_fin(g, rmk4, kros)
            rms16 = stp.tile([128, 16], F32, tag="rms16")
            qros = []
            for i in range(4):
                q_sub(i, rms16, i, qros, lead=True)
            q_fin(0, rms16, qros)

            # ---- attention: exp spine + drip-fed side work -----------
            # Two FIFO queues keep non-spine work out of the scores->exp
            # chain's way: wpv (PV sweeps + normalize/out_proj, ordered to
            # respect the single oraw psum slot) and wq (next block's q
            # projection chain).
            from collections import deque
            wpv, wq_ = deque(), deque()

            def pump(q, n):
                for _ in range(n):
                    if not q:
                        return
                    q.popleft()()

            def pv_group(oraw_box, pts, jc, subs):
                def run():
                    if not oraw_box:
                        oraw_box.append(
                            ps.tile([128, 8, 128], F32, tag="oraw", name="oraw")
                        )
                    oraw = oraw_box[0]
                    for s2, sub in enumerate(subs):
                        for hd in range(4):
                            nc.tensor.matmul(
                                oraw[:, s2 * 4 + hd, 0:65],
                                lhsT=pts[jc][:, hd, sub * 128:(sub + 1) * 128],
                                rhs=v_sb[:, jc, 0:65],
                                start=(jc == 0), stop=(jc == 15),
                            )
                return run

            state = {}  # per-ic boxes

            for ic in range(NIC):
                isl = slice(ic * 512, (ic + 1) * 512)
                pts = []
                state[ic] = dict(pts=pts, o0=[], o1=[], oT0=[], oT1=[])
                # previous block's finish (both halves) + second PV half
                if ic > 0:
                    pv = state[ic - 1]
                    wpv.append(
                        (lambda p: lambda: norm_half(0, p["o0"][0], p["oT0"]))(pv)
                    )
                    wpv.append(
                        (lambda p, i: lambda: outproj_sub(i * 4 + 0, p["oT0"][0]))(pv, ic - 1)
                    )
                    wpv.append(
                        (lambda p, i: lambda: outproj_sub(i * 4 + 1, p["oT0"][1]))(pv, ic - 1)
                    )
                    for jc in range(16):
                        wpv.append(pv_group(pv["o1"], pv["pts"], jc, (2, 3)))
                    wpv.append(
                        (lambda p: lambda: norm_half(1, p["o1"][0], p["oT1"]))(pv)
                    )
                    wpv.append(
                        (lambda p, i: lambda: outproj_sub(i * 4 + 2, p["oT1"][0]))(pv, ic - 1)
                    )
                    wpv.append(
                        (lambda p, i: lambda: outproj_sub(i * 4 + 3, p["oT1"][1]))(pv, ic - 1)
                    )
                # next block's q projection chain
                if ic < 3:
                    nrms = stp.tile([128, 16], F32, tag="rms16")
                    nqros = []
                    for i in range(4):
                        sc = 4 * (ic + 1) + i
                        pq_box = []
                        for ko0 in (0, 2, 4, 6):
                            wq_.append(
                                (lambda s, b, k: lambda: q_sub_mm(s, b, k))(sc, pq_box, ko0)
                            )
                        wq_.append(
                            (lambda s, b, i2: lambda: q_sub_tail(s, nrms, i2, nqros, b[0]))(sc, pq_box, i)
                        )

                for jc in range(16):
                    pT_t = ptp.tile([128, 4, 512], BF16, tag="pT")
                    pts.append(pT_t)
                    if use_mask:
                        mkt = mp.tile([128, 512], F32, tag="mkt")
                        nc.sync.dma_start(out=mkt[:], in_=mk[jc][:, isl])
                    for pair in range(2):
                        pss = ps.tile([128, 2, 512], F32,
                                      tag=("scA" if pair == 0 else "scB"))
                        for hh in range(2):
                            rows = slice(64 * hh, 64 * hh + 64)
                            nc.tensor.matmul(
                                pss[:, hh, :],
                                lhsT=kT[rows, jc * 128:(jc + 1) * 128],
                                rhs=qT[rows, pair, isl],
                                start=True, stop=True,
                            )
                        if use_mask:
                            sm = mp.tile([128, 2, 512], F32, tag="sm")
                            nc.vector.scalar_tensor_tensor(
                                sm[:], pss[:], 0.125,
                                mkt[:, None, :].to_broadcast((128, 2, 512)),
                                AL.mult, AL.add,
                            )
                            nc.scalar.activation(
                                pT_t[:, 2 * pair:2 * pair + 2, :], sm[:], AF.Exp
                            )
                        else:
                            nc.scalar.activation(
                                pT_t[:, 2 * pair:2 * pair + 2, :], pss[:], AF.Exp,
                                scale=0.125,
                            )
                    # this block's first-half PV, lagged behind the spine
                    wpv.append(pv_group(state[ic]["o0"], pts, jc, (0, 1)))
                    pump(wpv, 4 if jc < 8 else 3)
                    if ic < 3 and jc == 9:
                        pump(wq_, len(wq_))
                        q_fin(ic + 1, nrms, nqros)
                    else:
                        pump(wq_, 3)

            # tail: finish last block (both halves)
            pv = state[3]
            pump(wpv, len(wpv))
            norm_half(0, pv["o0"][0], pv["oT0"])
            outproj_sub(12, pv["oT0"][0])
            outproj_sub(13, pv["oT0"][1])
            for jc in range(16):
                pv_group(pv["o1"], pv["pts"], jc, (2, 3))()
            norm_half(1, pv["o1"][0], pv["oT1"])
            outproj_sub(14, pv["oT1"][0])
            outproj_sub(15, pv["oT1"][1])

    nc.compile()
    return nc


def _get(use_mask: bool, use_bias: bool = False):
    key = (use_mask, use_bias)
    if key not in _cache:
        _cache[key] = _build(use_mask, use_bias)
    return _cache[key]


def _host_prep(hidden_state, attention_mask, Wq, bq, Wk, bk, Wv, bv, Wo,
               use_mask, use_bias):
    """Build the 8 per-core input maps."""
    half_q, half_k = HID // 2, (HKV * D) // 2  # 512, 128
    inv_q = ROPE_BASE ** (-np.arange(half_q, dtype=np.float64) / half_q)
    inv_k = ROPE_BASE ** (-np.arange(half_k, dtype=np.float64) / half_k)
    s_idx = np.arange(S, dtype=np.float64)
    ang_q = inv_q[:, None] * s_idx[None, :]  # [512, S] freq-major
    ang_k = inv_k[:, None] * s_idx[None, :]  # [128, S]
    cos_q, sin_q = np.cos(ang_q), np.sin(ang_q)
    cos_k, sin_k = np.cos(ang_k), np.sin(ang_k)

    in_maps = []
    for core in range(8):
        b, j = core // 4, core % 4
        own_q = np.arange(j * 256, (j + 1) * 256)
        par_q = own_q + 512 if j < 2 else own_q - 512
        fidx_q = own_q if j < 2 else own_q - 512
        sign = -1.0 if j < 2 else 1.0
        own_k = np.arange(j * 64, (j + 1) * 64)
        par_k = own_k + 128 if j < 2 else own_k - 128
        fidx_k = own_k if j < 2 else own_k - 128

        hTc = np.ascontiguousarray(hidden_state[b].T).astype(NB).reshape(8, 128, S)
        wq_c = np.concatenate([Wq[:, own_q], Wq[:, par_q]], axis=1)
        wq_c = wq_c.astype(NB).reshape(8, 128, QSEL)
        wk_c = np.concatenate([Wk[:, own_k], Wk[:, par_k]], axis=1)
        wk_c = wk_c.astype(NB).reshape(8, 128, 128)
        wv_c = Wv[:, own_k].astype(NB).reshape(8, 128, 64)
        wo_c = Wo[j * 256:(j + 1) * 256, :].astype(NB).reshape(2, 128, HID)
        # natural-layout cos/sin: [sc, seq128, {cos, signed sin}, feat]
        csq_c = np.stack(
            [cos_q[fidx_q].T, (sign * sin_q[fidx_q]).T], axis=1
        )  # [S, 2, 256]
        csq_c = csq_c.astype(NB).reshape(NSC, 128, 2, 256)
        csk_c = np.stack(
            [cos_k[fidx_k].T, (sign * sin_k[fidx_k]).T], axis=1
        )  # [S, 2, 64]
        csk_c = csk_c.astype(NB).reshape(NSC, 128, 2, 64)

        m = {
            "hT": hTc, "wq": wq_c, "wk": wk_c, "wv": wv_c, "wo": wo_c,
            "csq": csq_c, "csk": csk_c,
        }
        if use_mask:
            mT = np.ascontiguousarray(attention_mask[b].T).astype(np.float32)
            m["mk"] = mT.reshape(NSC, 128, S)
        if use_bias:
            m["brq"] = np.concatenate([bq[own_q], bq[par_q]]).astype(NB).reshape(1, QSEL)
            m["brk"] = np.concatenate([bk[own_k], bk[par_k]]).astype(NB).reshape(1, 128)
            m["brv"] = bv[own_k].astype(NB).reshape(1, 64)
        in_maps.append(m)
    return in_maps


def kernel(hidden_state, attention_mask, Wq, bq, Wk, bk, Wv, bv, Wo, bo):
    from concourse.bass_utils import run_bass_kernel_spmd

    hidden_state = np.asarray(hidden_state, dtype=np.float32)
    attention_mask = np.asarray(attention_mask, dtype=np.float32)
    Wq, bq = np.asarray(Wq, np.float32), np.asarray(bq, np.float32)
    Wk, bk = np.asarray(Wk, np.float32), np.asarray(bk, np.float32)
    Wv, bv = np.asarray(Wv, np.float32), np.asarray(bv, np.float32)
    Wo, bo = np.asarray(Wo, np.float32), np.asarray(bo, np.float32)
    use_mask = bool(np.any(attention_mask))
    use_bias = bool(np.any(bq) or np.any(bk) or np.any(bv))
    nc = _get(use_mask, use_bias)
    in_maps = _host_prep(
        hidden_state, attention_mask, Wq, bq, Wk, bk, Wv, bv, Wo,
        use_mask, use_bias,
    )
    res = run_bass_kernel_spmd(nc, in_maps, list(range(8)))
    out = np.zeros((B, S, HID), dtype=np.float32)
    for core in range(8):
        out[core // 4] += res.results[core]["y"].reshape(S, HID)
    out += bo[None, None, :]
    return out


# revision 18
# speedup vs baseline: 1.0211x; 1.0211x over previous
"""GQA attention kernel for Trainium2, 8 NeuronCores.

Sharding: data-parallel over batch (B=2) x tensor-parallel over KV heads
(HKV=4) -> 8 cores.  Core c handles batch b=c//4, kv-head j=c%4 with its
G=4 query heads.  out_proj is row-parallel; partials are reduced on host.

Layout strategy (v2):
  - Projections in NATURAL orientation (out[seq, feat]): lhsT = hiddenT
    chunk, rhs = W chunk.  RoPE and rmsnorm then operate along the free
    dim (cheap DVE/Pool ops, no partition reductions).
  - rsqrt for rmsnorm is exp(-0.5*ln(x)) on ACT: both funcs live in the
    natural_log_exp_and_others activation table together with the softmax
    Exp, so the ACT engine never reloads its table.
  - qT / kT for the scores matmuls are produced by DMA-transpose
    (crossbar) instructions; kT's row-64..127 duplicate is folded into
    the same transpose by duplicating kn columns beforehand.
  - scoresT[key, q] = kT^T @ qT per head, exp on ACT (the hard floor:
    ~110us of exp at 0.833 ns/elem), probabilities pT kept in SBUF for a
    full 512-q block.
  - PV in flipped orientation: out[q, d+1] with lhsT = pT chunk,
    rhs = v (with ones column -> denominator lands as column 64).  N=65
    per matmul instead of 512 -> half the PE rows of the baseline.
    Normalization is a per-partition reciprocal + broadcast multiply.
  - oT via DMA-transpose feeds a row-parallel out_proj; partials DMA'd
    per 128-row chunk.
PSUM budget (8 banks): pq 1 | scoresA 2 | scoresB 2 | oraw 2 | y 1.
The lead-in k/v/q chains round-robin across all five slots.
"""

import numpy as np
import ml_dtypes

import concourse.bacc as bacc
import concourse.mybir as mybir
from concourse.tile import TileContext

BF16 = mybir.dt.bfloat16
F32 = mybir.dt.float32
AL = mybir.AluOpType
AF = mybir.ActivationFunctionType
AX = mybir.AxisListType

B, S, HID = 2, 2048, 1024
H, HKV, D = 16, 4, 64
G = H // HKV          # 4 query heads per kv head
QSEL = 2 * G * D      # 512: own 256 cols + rope-partner 256 cols
ROPE_BASE = 10000.0
EPS = float(np.finfo(np.float32).eps)
NSC = S // 128        # 16 seq chunks
NIC = 4               # 512-wide q blocks

NB = ml_dtypes.bfloat16

_cache: dict = {}


def _build(use_mask: bool, use_bias: bool):
    nc = bacc.Bacc("TRN2", target_bir_lowering=False)

    hT = nc.dram_tensor("hT", [8, 128, S], BF16, kind="ExternalInput")
    wq = nc.dram_tensor("wq", [8, 128, QSEL], BF16, kind="ExternalInput")
    wk = nc.dram_tensor("wk", [8, 128, 128], BF16, kind="ExternalInput")
    wv = nc.dram_tensor("wv", [8, 128, 64], BF16, kind="ExternalInput")
    wo = nc.dram_tensor("wo", [2, 128, HID], BF16, kind="ExternalInput")
    csq = nc.dram_tensor("csq", [NSC, 128, 2, 256], BF16, kind="ExternalInput")
    csk = nc.dram_tensor("csk", [NSC, 128, 2, 64], BF16, kind="ExternalInput")
    y = nc.dram_tensor("y", [NSC, 128, HID], F32, kind="ExternalOutput")
    mk = (
        nc.dram_tensor("mk", [NSC, 128, S], F32, kind="ExternalInput")
        if use_mask
        else None
    )
    if use_bias:
        brq = nc.dram_tensor("brq", [1, QSEL], BF16, kind="ExternalInput")
        brk = nc.dram_tensor("brk", [1, 128], BF16, kind="ExternalInput")
        brv = nc.dram_tensor("brv", [1, 64], BF16, kind="ExternalInput")

    with TileContext(nc) as tc:
        with (
            tc.tile_pool(name="const", bufs=1) as cp,
            tc.tile_pool(name="proj", bufs=1) as pj,
            tc.tile_pool(name="rt", bufs=3) as rt,
            tc.tile_pool(name="ro", bufs=6) as rop,
            tc.tile_pool(name="stat", bufs=3) as stp,
            tc.tile_pool(name="pT", bufs=20) as ptp,
            tc.tile_pool(name="onat", bufs=3) as onp_,
            tc.tile_pool(name="oTp", bufs=3) as otp,
            tc.tile_pool(name="ysb", bufs=2) as yp,
            tc.tile_pool(name="maskp", bufs=3) as mp,
            tc.tile_pool(name="ps", bufs=1, space="PSUM") as ps,
        ):
            # ---- persistent tiles ------------------------------------
            wo_sb = cp.tile([128, 2, HID], BF16)
            for cc in range(2):
                nc.sync.dma_start(out=wo_sb[:, cc, :], in_=wo[cc])
            v_sb = cp.tile([128, NSC, 66], BF16)
            nc.vector.memset(v_sb[:, :, 64:65], 1.0)
            eps_sb = cp.tile([128, 1], F32)
            nc.vector.memset(eps_sb[:], EPS)
            qT = cp.tile([128, 2, S], BF16)
            kT = cp.tile([128, S], BF16)

            # ---- projection-phase constants --------------------------
            # small weights first so the first k/v matmuls start early
            wk_sb = pj.tile([128, 8, 128], BF16)
            nc.sync.dma_start(out=wk_sb[:], in_=wk[:].rearrange("a b c -> b a c"))
            wv_sb = pj.tile([128, 8, 64], BF16)
            nc.sync.dma_start(out=wv_sb[:], in_=wv[:].rearrange("a b c -> b a c"))
            csk_sb = pj.tile([128, NSC, 2, 64], BF16)
            nc.sync.dma_start(out=csk_sb[:], in_=csk[:].rearrange("a b c d -> b a c d"))
            hT_sb = pj.tile([128, 8, S], BF16)
            for ko in range(8):
                nc.sync.dma_start(out=hT_sb[:, ko, :], in_=hT[ko])
            wq_sb = pj.tile([128, 8, QSEL], BF16)
            nc.sync.dma_start(out=wq_sb[:], in_=wq[:].rearrange("a b c -> b a c"))
            csq_sb = pj.tile([128, NSC, 2, 256], BF16)
            nc.sync.dma_start(out=csq_sb[:], in_=csq[:].rearrange("a b c d -> b a c d"))
            if use_bias:
                ones1 = cp.tile([1, 128], BF16)
                nc.vector.memset(ones1[:], 1.0)
                brq_sb = cp.tile([1, QSEL], BF16)
                nc.sync.dma_start(out=brq_sb[:], in_=brq[:])
                brk_sb = cp.tile([1, 128], BF16)
                nc.sync.dma_start(out=brk_sb[:], in_=brk[:])
                brv_sb = cp.tile([1, 64], BF16)
                nc.sync.dma_start(out=brv_sb[:], in_=brv[:])

            # PSUM slot round-robin for the lead-in projection chains
            SLOTS = ["scA", "scB", "oraw", "py", "pq"]
            slot_i = [0]

            def next_slot():
                s = SLOTS[slot_i[0] % len(SLOTS)]
                slot_i[0] += 1
                return s

            def rsqrt_batch(rm, tag):
                """rm: [128, n] f32 sums of squares/64 -> (rm+eps)^-0.5 via
                exp(-0.5 * ln(rm + eps)); stays in the exp table set."""
                n = rm.shape[1]
                ln_t = stp.tile([128, n], F32, tag=tag + "_ln")
                nc.scalar.activation(ln_t[:], rm[:], AF.Ln, bias=eps_sb[:])
                rc = stp.tile([128, n], F32, tag=tag + "_rc")
                nc.scalar.activation(rc[:], ln_t[:], AF.Exp, scale=-0.5)
                return rc

            def kv_sub(sc, rmk4, i, kros):
                ssl = slice(sc * 128, (sc + 1) * 128)
                pk = ps.tile([128, 2, 64], F32, tag=next_slot())
                pv = ps.tile([128, 64], F32, tag=next_slot())
                for ko in range(8):
                    st, sp = ko == 0, (ko == 7 and not use_bias)
                    nc.tensor.matmul(
                        pk[:], lhsT=hT_sb[:, ko, ssl], rhs=wk_sb[:, ko, :],
                        start=st, stop=sp,
                    )
                for ko in range(8):
                    st, sp = ko == 0, (ko == 7 and not use_bias)
                    nc.tensor.matmul(
                        pv[:], lhsT=hT_sb[:, ko, ssl], rhs=wv_sb[:, ko, :],
                        start=st, stop=sp,
                    )
                if use_bias:
                    nc.tensor.matmul(pk[:], lhsT=ones1[:], rhs=brk_sb[:],
                                     start=False, stop=True)
                    nc.tensor.matmul(pv[:], lhsT=ones1[:], rhs=brv_sb[:],
                                     start=False, stop=True)
                t12k = rt.tile([128, 2, 64], BF16, tag="t12k")
                nc.vector.tensor_tensor(t12k[:], pk[:], csk_sb[:, sc, :, :], AL.mult)
                kro = rop.tile([128, 64], BF16, tag="kro")
                nc.gpsimd.tensor_tensor(kro[:], t12k[:, 0, :], t12k[:, 1, :], AL.add)
                kros.append(kro)
                sqk = rt.tile([128, 64], BF16, tag="sqk")
                nc.gpsimd.scalar_tensor_tensor(
                    sqk[:], kro[:], 1.0 / 64.0, kro[:], AL.mult, AL.mult
                )
                nc.vector.tensor_reduce(rmk4[:, i:i + 1], sqk[:], AX.X, AL.add)
                nc.vector.tensor_copy(v_sb[:, sc, 0:64], pv[:])

            def kv_fin(g, rmk4, kros):
                rck = rsqrt_batch(rmk4, "rck")
                for i in range(4):
                    sc = 4 * g + i
                    kn2 = rt.tile([128, 2, 64], BF16, tag="kn2")
                    nc.vector.tensor_scalar_mul(kn2[:, 0, :], kros[i][:], rck[:, i:i + 1])
                    nc.gpsimd.tensor_copy(kn2[:, 1, :], kn2[:, 0, :])
                    nc.sync.dma_start_transpose(
                        out=kT[:, sc * 128:(sc + 1) * 128], in_=kn2[:]
                    )

            def q_sub(sc, rms16, i, qros, lead=False):
                ssl = slice(sc * 128, (sc + 1) * 128)
                pq = ps.tile([128, 2, 256], F32, tag=(next_slot() if lead else "pq"))
                for ko in range(8):
                    st, sp = ko == 0, (ko == 7 and not use_bias)
                    nc.tensor.matmul(
                        pq[:], lhsT=hT_sb[:, ko, ssl], rhs=wq_sb[:, ko, :],
                        start=st, stop=sp,
                    )
                if use_bias:
                    nc.tensor.matmul(pq[:], lhsT=ones1[:], rhs=brq_sb[:],
                                     start=False, stop=True)
                q_sub_tail(sc, rms16, i, qros, pq)

            def q_sub_mm(sc, pq_box, ko0, lead=False):
                """two accumulation matmuls of the q projection for chunk sc"""
                ssl = slice(sc * 128, (sc + 1) * 128)
                if ko0 == 0:
                    pq_box.append(
                        ps.tile([128, 2, 256], F32,
                                tag=(next_slot() if lead else "pq"), name="pq")
                    )
                pq = pq_box[0]
                for ko in (ko0, ko0 + 1):
                    st = ko == 0
                    sp = ko == 7 and not use_bias
                    nc.tensor.matmul(
                        pq[:], lhsT=hT_sb[:, ko, ssl], rhs=wq_sb[:, ko, :],
                        start=st, stop=sp,
                    )
                if ko0 == 6 and use_bias:
                    nc.tensor.matmul(pq[:], lhsT=ones1[:], rhs=brq_sb[:],
                                     start=False, stop=True)

            def q_sub_tail(sc, rms16, i, qros, pq):
                t12 = rt.tile([128, 2, 256], BF16, tag="t12")
                nc.vector.tensor_tensor(t12[:], pq[:], csq_sb[:, sc, :, :], AL.mult)
                qro = rop.tile([128, 4, 64], BF16, tag="qro")
                nc.gpsimd.tensor_tensor(
                    qro[:].rearrange("p h d -> p (h d)"), t12[:, 0, :], t12[:, 1, :],
                    AL.add,
                )
                qros.append(qro)
                sqq = rt.tile([128, 4, 64], BF16, tag="sqq")
                nc.gpsimd.scalar_tensor_tensor(
                    sqq[:], qro[:], 1.0 / 64.0, qro[:], AL.mult, AL.mult
                )
                nc.vector.tensor_reduce(rms16[:, 4 * i:4 * i + 4], sqq[:], AX.X, AL.add)

            def q_fin(ic, rms16, qros):
                rcq = rsqrt_batch(rms16, "rcq")
                for i in range(4):
                    sc = 4 * ic + i
                    qn = rt.tile([128, 4, 64], BF16, tag="qn")
                    nc.vector.tensor_tensor(
                        qn[:], qros[i][:],
                        rcq[:, 4 * i:4 * i + 4, None].to_broadcast((128, 4, 64)),
                        AL.mult,
                    )
                    nc.sync.dma_start_transpose(
                        out=qT[:, :, sc * 128:(sc + 1) * 128], in_=qn[:]
                    )

            def norm_half(h, oraw, box):
                # oraw: [128, 8, 128] psum, slices (s2, hd) at s2*4+hd, col 64 = denom
                rcp = stp.tile([128, 8], F32, tag="rcp")
                nc.vector.reciprocal(rcp[:], oraw[:, :, 64:65])
                for s2 in range(2):
                    onat = onp_.tile([128, 4, 64], BF16, tag="onat")
                    nc.vector.tensor_tensor(
                        onat[:], oraw[:, s2 * 4:(s2 + 1) * 4, 0:64],
                        rcp[:, s2 * 4:(s2 + 1) * 4, None].to_broadcast((128, 4, 64)),
                        AL.mult,
                    )
                    oTt = otp.tile([128, 2, 128], BF16, tag="oTt")
                    nc.sync.dma_start_transpose(out=oTt[:], in_=onat[:])
                    box.append(oTt)

            def outproj_sub(gc, oTt):
                ysb = yp.tile([128, HID], F32, tag="ysb")
                for ec in range(2):
                    py = ps.tile([128, 512], F32, tag="py")
                    for cc in range(2):
                        nc.tensor.matmul(
                            py[:], lhsT=oTt[:, cc, :],
                            rhs=wo_sb[:, cc, ec * 512:(ec + 1) * 512],
                            start=(cc == 0), stop=(cc == 1),
                        )
                    nc.vector.tensor_copy(ysb[:, ec * 512:(ec + 1) * 512], py[:])
                nc.sync.dma_start(out=y[gc], in_=ysb[:])

            # ---- lead-in: k/v for all chunks, then q for ic 0 --------
            for g in range(4):
                rmk4 = stp.tile([128, 4], F32, tag="rmk4")
                kros = []
                for i in range(4):
                    kv_sub(4 * g + i, rmk4, i, kros)
                kv_fin(g, rmk4, kros)
            rms16 = stp.tile([128, 16], F32, tag="rms16")
            qros = []
            for i in range(4):
                q_sub(i, rms16, i, qros, lead=True)
            q_fin(0, rms16, qros)

            # ---- attention: exp spine + drip-fed side work -----------
            # Two FIFO queues keep non-spine work out of the scores->exp
            # chain's way: wpv (PV sweeps + normalize/out_proj, ordered to
            # respect the single oraw psum slot) and wq (next block's q
            # projection chain).
            from collections import deque
            wpv, wq_ = deque(), deque()

            def pump(q, n):
                for _ in range(n):
                    if not q:
                        return
                    q.popleft()()

            def pv_group(oraw_box, pts, jc, subs):
                def run():
                    if not oraw_box:
                        oraw_box.append(
                            ps.tile([128, 8, 128], F32, tag="oraw", name="oraw")
                        )
                    oraw = oraw_box[0]
                    for s2, sub in enumerate(subs):
                        for hd in range(4):
                            nc.tensor.matmul(
                                oraw[:, s2 * 4 + hd, 0:65],
                                lhsT=pts[jc][:, hd, sub * 128:(sub + 1) * 128],
                                rhs=v_sb[:, jc, 0:65],
                                start=(jc == 0), stop=(jc == 15),
                            )
                return run

            state = {}  # per-ic boxes

            for ic in range(NIC):
                isl = slice(ic * 512, (ic + 1) * 512)
                pts = []
                state[ic] = dict(pts=pts, o0=[], o1=[], oT0=[], oT1=[])
                # previous block's finish (both halves) + second PV half
                if ic > 0:
                    pv = state[ic - 1]
                    wpv.append(
                        (lambda p: lambda: norm_half(0, p["o0"][0], p["oT0"]))(pv)
                    )
                    wpv.append(
                        (lambda p, i: lambda: outproj_sub(i * 4 + 0, p["oT0"][0]))(pv, ic - 1)
                    )
                    wpv.append(
                        (lambda p, i: lambda: outproj_sub(i * 4 + 1, p["oT0"][1]))(pv, ic - 1)
                    )
                    for jc in range(16):
                        wpv.append(pv_group(pv["o1"], pv["pts"], jc, (2, 3)))
                    wpv.append(
                        (lambda p: lambda: norm_half(1, p["o1"][0], p["oT1"]))(pv)
                    )
                    wpv.append(
                        (lambda p, i: lambda: outproj_sub(i * 4 + 2, p["oT1"][0]))(pv, ic - 1)
                    )
                    wpv.append(
                        (lambda p, i: lambda: outproj_sub(i * 4 + 3, p["oT1"][1]))(pv, ic - 1)
                    )
                # next block's q projection chain
                if ic < 3:
                    nrms = stp.tile([128, 16], F32, tag="rms16")
                    nqros = []
                    for i in range(4):
                        sc = 4 * (ic + 1) + i
                        pq_box = []
                        for ko0 in (0, 2, 4, 6):
                            wq_.append(
                                (lambda s, b, k: lambda: q_sub_mm(s, b, k))(sc, pq_box, ko0)
                            )
                        wq_.append(
                            (lambda s, b, i2: lambda: q_sub_tail(s, nrms, i2, nqros, b[0]))(sc, pq_box, i)
                        )

                for jc in range(16):
                    pT_t = ptp.tile([128, 4, 512], BF16, tag="pT")
                    pts.append(pT_t)
                    if use_mask:
                        mkt = mp.tile([128, 512], F32, tag="mkt")
                        nc.sync.dma_start(out=mkt[:], in_=mk[jc][:, isl])
                    for pair in range(2):
                        pss = ps.tile([128, 2, 512], F32,
                                      tag=("scA" if pair == 0 else "scB"))
                        for hh in range(2):
                            rows = slice(64 * hh, 64 * hh + 64)
                            nc.tensor.matmul(
                                pss[:, hh, :],
                                lhsT=kT[rows, jc * 128:(jc + 1) * 128],
                                rhs=qT[rows, pair, isl],
                                start=True, stop=True,
                            )
                        if use_mask:
                            sm = mp.tile([128, 2, 512], F32, tag="sm")
                            nc.vector.scalar_tensor_tensor(
                                sm[:], pss[:], 0.125,
                                mkt[:, None, :].to_broadcast((128, 2, 512)),
                                AL.mult, AL.add,
                            )
                            nc.scalar.activation(
                                pT_t[:, 2 * pair:2 * pair + 2, :], sm[:], AF.Exp
                            )
                        else:
                            nc.scalar.activation(
                                pT_t[:, 2 * pair:2 * pair + 2, :], pss[:], AF.Exp,
                                scale=0.125,
                            )
                    # this block's first-half PV, lagged behind the spine
                    wpv.append(pv_group(state[ic]["o0"], pts, jc, (0, 1)))
                    pump(wpv, 4 if jc < 8 else 3)
                    if ic < 3 and jc == 9:
                        pump(wq_, len(wq_))
                        q_fin(ic + 1, nrms, nqros)
                    else:
                        pump(wq_, 3)

            # tail: finish last block (both halves)
            pv = state[3]
            pump(wpv, len(wpv))
            norm_half(0, pv["o0"][0], pv["oT0"])
            outproj_sub(12, pv["oT0"][0])
            outproj_sub(13, pv["oT0"][1])
            for jc in range(16):
                pv_group(pv["o1"], pv["pts"], jc, (2, 3))()
            norm_half(1, pv["o1"][0], pv["oT1"])
            outproj_sub(14, pv["oT1"][0])
            outproj_sub(15, pv["oT1"][1])

    nc.compile()
    return nc


def _get(use_mask: bool, use_bias: bool = False):
    key = (use_mask, use_bias)
    if key not in _cache:
        _cache[key] = _build(use_mask, use_bias)
    return _cache[key]


def _host_prep(hidden_state, attention_mask, Wq, bq, Wk, bk, Wv, bv, Wo,
               use_mask, use_bias):
    """Build the 8 per-core input maps."""
    half_q, half_k = HID // 2, (HKV * D) // 2  # 512, 128
    inv_q = ROPE_BASE ** (-np.arange(half_q, dtype=np.float64) / half_q)
    inv_k = ROPE_BASE ** (-np.arange(half_k, dtype=np.float64) / half_k)
    s_idx = np.arange(S, dtype=np.float64)
    ang_q = inv_q[:, None] * s_idx[None, :]  # [512, S] freq-major
    ang_k = inv_k[:, None] * s_idx[None, :]  # [128, S]
    cos_q, sin_q = np.cos(ang_q), np.sin(ang_q)
    cos_k, sin_k = np.cos(ang_k), np.sin(ang_k)

    in_maps = []
    for core in range(8):
        b, j = core // 4, core % 4
        own_q = np.arange(j * 256, (j + 1) * 256)
        par_q = own_q + 512 if j < 2 else own_q - 512
        fidx_q = own_q if j < 2 else own_q - 512
        sign = -1.0 if j < 2 else 1.0
        own_k = np.arange(j * 64, (j + 1) * 64)
        par_k = own_k + 128 if j < 2 else own_k - 128
        fidx_k = own_k if j < 2 else own_k - 128

        hTc = np.ascontiguousarray(hidden_state[b].T).astype(NB).reshape(8, 128, S)
        wq_c = np.concatenate([Wq[:, own_q], Wq[:, par_q]], axis=1)
        wq_c = wq_c.astype(NB).reshape(8, 128, QSEL)
        wk_c = np.concatenate([Wk[:, own_k], Wk[:, par_k]], axis=1)
        wk_c = wk_c.astype(NB).reshape(8, 128, 128)
        wv_c = Wv[:, own_k].astype(NB).reshape(8, 128, 64)
        wo_c = Wo[j * 256:(j + 1) * 256, :].astype(NB).reshape(2, 128, HID)
        # natural-layout cos/sin: [sc, seq128, {cos, signed sin}, feat]
        csq_c = np.stack(
            [cos_q[fidx_q].T, (sign * sin_q[fidx_q]).T], axis=1
        )  # [S, 2, 256]
        csq_c = csq_c.astype(NB).reshape(NSC, 128, 2, 256)
        csk_c = np.stack(
            [cos_k[fidx_k].T, (sign * sin_k[fidx_k]).T], axis=1
        )  # [S, 2, 64]
        csk_c = csk_c.astype(NB).reshape(NSC, 128, 2, 64)

        m = {
            "hT": hTc, "wq": wq_c, "wk": wk_c, "wv": wv_c, "wo": wo_c,
            "csq": csq_c, "csk": csk_c,
        }
        if use_mask:
            mT = np.ascontiguousarray(attention_mask[b].T).astype(np.float32)
            m["mk"] = mT.reshape(NSC, 128, S)
        if use_bias:
            m["brq"] = np.concatenate([bq[own_q], bq[par_q]]).astype(NB).reshape(1, QSEL)
            m["brk"] = np.concatenate([bk[own_k], bk[par_k]]).astype(NB).reshape(1, 128)
            m["brv"] = bv[own_k].astype(NB).reshape(1, 64)
        in_maps.append(m)
    return in_maps


def kernel(hidden_state, attention_mask, Wq, bq, Wk, bk, Wv, bv, Wo, bo):
    from concourse.bass_utils import run_bass_kernel_spmd

    hidden_state = np.asarray(hidden_state, dtype=np.float32)
    attention_mask = np.asarray(attention_mask, dtype=np.float32)
    Wq, bq = np.asarray(Wq, np.float32), np.asarray(bq, np.float32)
    Wk, bk = np.asarray(Wk, np.float32), np.asarray(bk, np.float32)
    Wv, bv = np.asarray(Wv, np.float32), np.asarray(bv, np.float32)
    Wo, bo = np.asarray(Wo, np.float32), np.asarray(bo, np.float32)
    use_mask = bool(np.any(attention_mask))
    use_bias = bool(np.any(bq) or np.any(bk) or np.any(bv))
    nc = _get(use_mask, use_bias)
    in_maps = _host_prep(
        hidden_state, attention_mask, Wq, bq, Wk, bk, Wv, bv, Wo,
        use_mask, use_bias,
    )
    res = run_bass_kernel_spmd(nc, in_maps, list(range(8)))
    out = np.zeros((B, S, HID), dtype=np.float32)
    for core in range(8):
        out[core // 4] += res.results[core]["y"].reshape(S, HID)
    out += bo[None, None, :]
    return out


# revision 23
# speedup vs baseline: 1.0536x; 1.0318x over previous
"""GQA attention kernel for Trainium2, 8 NeuronCores.

Sharding: data-parallel over batch (B=2) x tensor-parallel over KV heads
(HKV=4) -> 8 cores.  Core c handles batch b=c//4, kv-head j=c%4 with its
G=4 query heads.  out_proj is row-parallel; partials are reduced on host.

Layout strategy (v2):
  - Projections in NATURAL orientation (out[seq, feat]): lhsT = hiddenT
    chunk, rhs = W chunk.  RoPE and rmsnorm then operate along the free
    dim (cheap DVE/Pool ops, no partition reductions).
  - rsqrt for rmsnorm is exp(-0.5*ln(x)) on ACT: both funcs live in the
    natural_log_exp_and_others activation table together with the softmax
    Exp, so the ACT engine never reloads its table.
  - qT / kT for the scores matmuls are produced by DMA-transpose
    (crossbar) instructions; kT's row-64..127 duplicate is folded into
    the same transpose by duplicating kn columns beforehand.
  - scoresT[key, q] = kT^T @ qT per head, exp on ACT (the hard floor:
    ~110us of exp at 0.833 ns/elem), probabilities pT kept in SBUF for a
    full 512-q block.
  - PV in flipped orientation: out[q, d+1] with lhsT = pT chunk,
    rhs = v (with ones column -> denominator lands as column 64).  N=65
    per matmul instead of 512 -> half the PE rows of the baseline.
    Normalization is a per-partition reciprocal + broadcast multiply.
  - oT via DMA-transpose feeds a row-parallel out_proj; partials DMA'd
    per 128-row chunk.
PSUM budget (8 banks): pq 1 | scoresA 2 | scoresB 2 | oraw 2 | y 1.
The lead-in k/v/q chains round-robin across all five slots.
"""

import numpy as np
import ml_dtypes

import concourse.bacc as bacc
import concourse.mybir as mybir
from concourse.tile import TileContext

BF16 = mybir.dt.bfloat16
F32 = mybir.dt.float32
AL = mybir.AluOpType
AF = mybir.ActivationFunctionType
AX = mybir.AxisListType

B, S, HID = 2, 2048, 1024
H, HKV, D = 16, 4, 64
G = H // HKV          # 4 query heads per kv head
QSEL = 2 * G * D      # 512: own 256 cols + rope-partner 256 cols
ROPE_BASE = 10000.0
EPS = float(np.finfo(np.float32).eps)
NSC = S // 128        # 16 seq chunks
NIC = 4               # 512-wide q blocks

NB = ml_dtypes.bfloat16

_cache: dict = {}


def _build(use_mask: bool, use_bias: bool):
    nc = bacc.Bacc("TRN2", target_bir_lowering=False)

    hT = nc.dram_tensor("hT", [8, 128, S], BF16, kind="ExternalInput")
    wq = nc.dram_tensor("wq", [8, 128, QSEL], BF16, kind="ExternalInput")
    wk = nc.dram_tensor("wk", [8, 128, 128], BF16, kind="ExternalInput")
    wv = nc.dram_tensor("wv", [8, 128, 64], BF16, kind="ExternalInput")
    wo = nc.dram_tensor("wo", [2, 128, HID], BF16, kind="ExternalInput")
    csq = nc.dram_tensor("csq", [NSC, 128, 2, 256], BF16, kind="ExternalInput")
    csk = nc.dram_tensor("csk", [NSC, 128, 2, 64], BF16, kind="ExternalInput")
    y = nc.dram_tensor("y", [NSC, 128, HID], F32, kind="ExternalOutput")
    mk = (
        nc.dram_tensor("mk", [NSC, 128, S], F32, kind="ExternalInput")
        if use_mask
        else None
    )
    if use_bias:
        brq = nc.dram_tensor("brq", [1, QSEL], BF16, kind="ExternalInput")
        brk = nc.dram_tensor("brk", [1, 128], BF16, kind="ExternalInput")
        brv = nc.dram_tensor("brv", [1, 64], BF16, kind="ExternalInput")

    with TileContext(nc) as tc:
        with (
            tc.tile_pool(name="const", bufs=1) as cp,
            tc.tile_pool(name="proj", bufs=1) as pj,
            tc.tile_pool(name="rt", bufs=3) as rt,
            tc.tile_pool(name="ro", bufs=6) as rop,
            tc.tile_pool(name="stat", bufs=3) as stp,
            tc.tile_pool(name="pT", bufs=20) as ptp,
            tc.tile_pool(name="onat", bufs=3) as onp_,
            tc.tile_pool(name="oTp", bufs=3) as otp,
            tc.tile_pool(name="ysb", bufs=2) as yp,
            tc.tile_pool(name="maskp", bufs=3) as mp,
            tc.tile_pool(name="ps", bufs=1, space="PSUM") as ps,
        ):
            # ---- persistent tiles ------------------------------------
            wo_sb = cp.tile([128, 2, HID], BF16)
            for cc in range(2):
                nc.sync.dma_start(out=wo_sb[:, cc, :], in_=wo[cc])
            v_sb = cp.tile([128, NSC, 66], BF16)
            nc.vector.memset(v_sb[:, :, 64:65], 1.0)
            eps_sb = cp.tile([128, 1], F32)
            nc.vector.memset(eps_sb[:], EPS)
            qT = cp.tile([128, 2, S], BF16)
            kT = cp.tile([128, S], BF16)

            # ---- projection-phase constants --------------------------
            # small weights first so the first k/v matmuls start early
            wk_sb = pj.tile([128, 8, 128], BF16)
            nc.sync.dma_start(out=wk_sb[:], in_=wk[:].rearrange("a b c -> b a c"))
            wv_sb = pj.tile([128, 8, 64], BF16)
            nc.sync.dma_start(out=wv_sb[:], in_=wv[:].rearrange("a b c -> b a c"))
            csk_sb = pj.tile([128, NSC, 2, 64], BF16)
            nc.sync.dma_start(out=csk_sb[:], in_=csk[:].rearrange("a b c d -> b a c d"))
            hT_sb = pj.tile([128, 8, S], BF16)
            for ko in range(8):
                nc.sync.dma_start(out=hT_sb[:, ko, :], in_=hT[ko])
            wq_sb = pj.tile([128, 8, QSEL], BF16)
            nc.sync.dma_start(out=wq_sb[:], in_=wq[:].rearrange("a b c -> b a c"))
            csq_sb = pj.tile([128, NSC, 2, 256], BF16)
            for qu in range(4):
                nc.sync.dma_start(
                    out=csq_sb[:, 4 * qu:4 * qu + 4, :, :],
                    in_=csq[4 * qu:4 * qu + 4].rearrange("a b c d -> b a c d"),
                )
            if use_bias:
                ones1 = cp.tile([1, 128], BF16)
                nc.vector.memset(ones1[:], 1.0)
                brq_sb = cp.tile([1, QSEL], BF16)
                nc.sync.dma_start(out=brq_sb[:], in_=brq[:])
                brk_sb = cp.tile([1, 128], BF16)
                nc.sync.dma_start(out=brk_sb[:], in_=brk[:])
                brv_sb = cp.tile([1, 64], BF16)
                nc.sync.dma_start(out=brv_sb[:], in_=brv[:])

            # PSUM slot round-robin for the lead-in projection chains
            SLOTS = ["scA", "scB", "oraw", "py", "pq"]
            slot_i = [0]

            def next_slot():
                s = SLOTS[slot_i[0] % len(SLOTS)]
                slot_i[0] += 1
                return s

            def rsqrt_batch(rm, tag):
                """rm: [128, n] f32 sums of squares/64 -> (rm+eps)^-0.5 via
                exp(-0.5 * ln(rm + eps)); stays in the exp table set."""
                n = rm.shape[1]
                ln_t = stp.tile([128, n], F32, tag=tag + "_ln")
                nc.scalar.activation(ln_t[:], rm[:], AF.Ln, bias=eps_sb[:])
                rc = stp.tile([128, n], F32, tag=tag + "_rc")
                nc.scalar.activation(rc[:], ln_t[:], AF.Exp, scale=-0.5)
                return rc

            def kv_sub(sc, rmk4, i, kros):
                ssl = slice(sc * 128, (sc + 1) * 128)
                pk = ps.tile([128, 2, 64], F32, tag=next_slot())
                pv = ps.tile([128, 64], F32, tag=next_slot())
                for ko in range(8):
                    st, sp = ko == 0, (ko == 7 and not use_bias)
                    nc.tensor.matmul(
                        pk[:], lhsT=hT_sb[:, ko, ssl], rhs=wk_sb[:, ko, :],
                        start=st, stop=sp,
                    )
                for ko in range(8):
                    st, sp = ko == 0, (ko == 7 and not use_bias)
                    nc.tensor.matmul(
                        pv[:], lhsT=hT_sb[:, ko, ssl], rhs=wv_sb[:, ko, :],
                        start=st, stop=sp,
                    )
                if use_bias:
                    nc.tensor.matmul(pk[:], lhsT=ones1[:], rhs=brk_sb[:],
                                     start=False, stop=True)
                    nc.tensor.matmul(pv[:], lhsT=ones1[:], rhs=brv_sb[:],
                                     start=False, stop=True)
                t12k = rt.tile([128, 2, 64], BF16, tag="t12k")
                nc.vector.tensor_tensor(t12k[:], pk[:], csk_sb[:, sc, :, :], AL.mult)
                kro = rop.tile([128, 64], BF16, tag="kro")
                nc.gpsimd.tensor_tensor(kro[:], t12k[:, 0, :], t12k[:, 1, :], AL.add)
                kros.append(kro)
                sqk = rt.tile([128, 64], BF16, tag="sqk")
                nc.gpsimd.scalar_tensor_tensor(
                    sqk[:], kro[:], 1.0 / 64.0, kro[:], AL.mult, AL.mult
                )
                nc.vector.tensor_reduce(rmk4[:, i:i + 1], sqk[:], AX.X, AL.add)
                nc.vector.tensor_copy(v_sb[:, sc, 0:64], pv[:])

            def kv_fin(g, rmk4, kros):
                rck = rsqrt_batch(rmk4, "rck")
                for i in range(4):
                    sc = 4 * g + i
                    kn2 = rt.tile([128, 2, 64], BF16, tag="kn2")
                    nc.vector.tensor_scalar_mul(kn2[:, 0, :], kros[i][:], rck[:, i:i + 1])
                    nc.gpsimd.tensor_copy(kn2[:, 1, :], kn2[:, 0, :])
                    nc.sync.dma_start_transpose(
                        out=kT[:, sc * 128:(sc + 1) * 128], in_=kn2[:]
                    )

            def q_sub(sc, rms16, i, qros, lead=False):
                ssl = slice(sc * 128, (sc + 1) * 128)
                pq = ps.tile([128, 2, 256], F32, tag=(next_slot() if lead else "pq"))
                for ko in range(8):
                    st, sp = ko == 0, (ko == 7 and not use_bias)
                    nc.tensor.matmul(
                        pq[:], lhsT=hT_sb[:, ko, ssl], rhs=wq_sb[:, ko, :],
                        start=st, stop=sp,
                    )
                if use_bias:
                    nc.tensor.matmul(pq[:], lhsT=ones1[:], rhs=brq_sb[:],
                                     start=False, stop=True)
                q_sub_tail(sc, rms16, i, qros, pq)

            def q_sub_mm(sc, pq_box, ko0, lead=False):
                """two accumulation matmuls of the q projection for chunk sc"""
                ssl = slice(sc * 128, (sc + 1) * 128)
                if ko0 == 0:
                    pq_box.append(
                        ps.tile([128, 2, 256], F32,
                                tag=(next_slot() if lead else "pq"), name="pq")
                    )
                pq = pq_box[0]
                for ko in (ko0, ko0 + 1):
                    st = ko == 0
                    sp = ko == 7 and not use_bias
                    nc.tensor.matmul(
                        pq[:], lhsT=hT_sb[:, ko, ssl], rhs=wq_sb[:, ko, :],
                        start=st, stop=sp,
                    )
                if ko0 == 6 and use_bias:
                    nc.tensor.matmul(pq[:], lhsT=ones1[:], rhs=brq_sb[:],
                                     start=False, stop=True)

            def q_sub_tail(sc, rms16, i, qros, pq):
                t12 = rt.tile([128, 2, 256], BF16, tag="t12")
                nc.vector.tensor_tensor(t12[:], pq[:], csq_sb[:, sc, :, :], AL.mult)
                qro = rop.tile([128, 4, 64], BF16, tag="qro")
                nc.gpsimd.tensor_tensor(
                    qro[:].rearrange("p h d -> p (h d)"), t12[:, 0, :], t12[:, 1, :],
                    AL.add,
                )
                qros.append(qro)
                sqq = rt.tile([128, 4, 64], BF16, tag="sqq")
                nc.gpsimd.scalar_tensor_tensor(
                    sqq[:], qro[:], 1.0 / 64.0, qro[:], AL.mult, AL.mult
                )
                nc.vector.tensor_reduce(rms16[:, 4 * i:4 * i + 4], sqq[:], AX.X, AL.add)

            def q_fin(ic, rms16, qros):
                rcq = rsqrt_batch(rms16, "rcq")
                for i in range(4):
                    sc = 4 * ic + i
                    qn = rt.tile([128, 4, 64], BF16, tag="qn")
                    nc.vector.tensor_tensor(
                        qn[:], qros[i][:],
                        rcq[:, 4 * i:4 * i + 4, None].to_broadcast((128, 4, 64)),
                        AL.mult,
                    )
                    nc.sync.dma_start_transpose(
                        out=qT[:, :, sc * 128:(sc + 1) * 128], in_=qn[:]
                    )

            def norm_half(h, oraw, box):
                # oraw: [128, 8, 128] psum, slices (s2, hd) at s2*4+hd, col 64 = denom
                rcp = stp.tile([128, 8], F32, tag="rcp")
                nc.vector.reciprocal(rcp[:], oraw[:, :, 64:65])
                for s2 in range(2):
                    onat = onp_.tile([128, 4, 64], BF16, tag="onat")
                    nc.vector.tensor_tensor(
                        onat[:], oraw[:, s2 * 4:(s2 + 1) * 4, 0:64],
                        rcp[:, s2 * 4:(s2 + 1) * 4, None].to_broadcast((128, 4, 64)),
                        AL.mult,
                    )
                    oTt = otp.tile([128, 2, 128], BF16, tag="oTt")
                    nc.sync.dma_start_transpose(out=oTt[:], in_=onat[:])
                    box.append(oTt)

            def norm_sub(oraw4, box):
                # oraw4: [128, 4, 128] psum, one q-sub chunk of 4 heads
                rcp = stp.tile([128, 4], F32, tag="rcp4")
                nc.vector.reciprocal(rcp[:], oraw4[:, :, 64:65])
                onat = onp_.tile([128, 4, 64], BF16, tag="onat")
                nc.vector.tensor_tensor(
                    onat[:], oraw4[:, :, 0:64],
                    rcp[:, :, None].to_broadcast((128, 4, 64)), AL.mult,
                )
                oTt = otp.tile([128, 2, 128], BF16, tag="oTt")
                nc.sync.dma_start_transpose(out=oTt[:], in_=onat[:])
                box.append(oTt)

            def pv_group1(oraw_box, tag, pts, jc, sub):
                def run():
                    if not oraw_box:
                        oraw_box.append(
                            ps.tile([128, 4, 128], F32, tag=tag, name="oraw4")
                        )
                    oraw = oraw_box[0]
                    for hd in range(4):
                        nc.tensor.matmul(
                            oraw[:, hd, 0:65],
                            lhsT=pts[jc][:, hd, sub * 128:(sub + 1) * 128],
                            rhs=v_sb[:, jc, 0:65],
                            start=(jc == 0), stop=(jc == 15),
                        )
                return run

            def outproj_sub(gc, oTt):
                ysb = yp.tile([128, HID], F32, tag="ysb")
                for ec in range(2):
                    py = ps.tile([128, 512], F32, tag="py")
                    for cc in range(2):
                        nc.tensor.matmul(
                            py[:], lhsT=oTt[:, cc, :],
                            rhs=wo_sb[:, cc, ec * 512:(ec + 1) * 512],
                            start=(cc == 0), stop=(cc == 1),
                        )
                    nc.vector.tensor_copy(ysb[:, ec * 512:(ec + 1) * 512], py[:])
                nc.sync.dma_start(out=y[gc], in_=ysb[:])

            # ---- lead-in: k/v group 0, q block 0, then k/v 1..3 ------
            # (the attention spine only needs kT chunk jc at step jc, so
            # later k/v groups can trail behind the first exps)
            def kv_group(g):
                rmk4 = stp.tile([128, 4], F32, tag="rmk4")
                kros = []
                for i in range(4):
                    kv_sub(4 * g + i, rmk4, i, kros)
                kv_fin(g, rmk4, kros)

            kv_group(0)
            rms16 = stp.tile([128, 16], F32, tag="rms16")
            qros = []
            for i in range(4):
                q_sub(i, rms16, i, qros, lead=True)
            q_fin(0, rms16, qros)
            for g in range(1, 4):
                kv_group(g)

            # ---- attention: exp spine + drip-fed side work -----------
            # Two FIFO queues keep non-spine work out of the scores->exp
            # chain's way: wpv (PV sweeps + normalize/out_proj, ordered to
            # respect the single oraw psum slot) and wq (next block's q
            # projection chain).
            from collections import deque
            wpv, wq_ = deque(), deque()

            def pump(q, n):
                for _ in range(n):
                    if not q:
                        return
                    q.popleft()()

            def pv_group(oraw_box, pts, jc, subs):
                def run():
                    if not oraw_box:
                        oraw_box.append(
                            ps.tile([128, 8, 128], F32, tag="oraw", name="oraw")
                        )
                    oraw = oraw_box[0]
                    for s2, sub in enumerate(subs):
                        for hd in range(4):
                            nc.tensor.matmul(
                                oraw[:, s2 * 4 + hd, 0:65],
                                lhsT=pts[jc][:, hd, sub * 128:(sub + 1) * 128],
                                rhs=v_sb[:, jc, 0:65],
                                start=(jc == 0), stop=(jc == 15),
                            )
                return run

            state = {}  # per-ic boxes

            for ic in range(NIC):
                isl = slice(ic * 512, (ic + 1) * 512)
                pts = []
                state[ic] = dict(pts=pts, o0=[], o1=[], o1b=[], oT0=[], oT1=[])
                # previous block's finish (both halves) + second PV half
                if ic > 0:
                    pv = state[ic - 1]
                    wpv.append(
                        (lambda p: lambda: norm_half(0, p["o0"][0], p["oT0"]))(pv)
                    )
                    wpv.append(
                        (lambda p, i: lambda: outproj_sub(i * 4 + 0, p["oT0"][0]))(pv, ic - 1)
                    )
                    wpv.append(
                        (lambda p, i: lambda: outproj_sub(i * 4 + 1, p["oT0"][1]))(pv, ic - 1)
                    )
                    for jc in range(16):
                        wpv.append(pv_group(pv["o1"], pv["pts"], jc, (2, 3)))
                    wpv.append(
                        (lambda p: lambda: norm_half(1, p["o1"][0], p["oT1"]))(pv)
                    )
                    wpv.append(
                        (lambda p, i: lambda: outproj_sub(i * 4 + 2, p["oT1"][0]))(pv, ic - 1)
                    )
                    wpv.append(
                        (lambda p, i: lambda: outproj_sub(i * 4 + 3, p["oT1"][1]))(pv, ic - 1)
                    )
                # next block's q projection chain
                if ic < 3:
                    nrms = stp.tile([128, 16], F32, tag="rms16")
                    nqros = []
                    for i in range(4):
                        sc = 4 * (ic + 1) + i
                        pq_box = []
                        for ko0 in (0, 2, 4, 6):
                            wq_.append(
                                (lambda s, b, k: lambda: q_sub_mm(s, b, k))(sc, pq_box, ko0)
                            )
                        wq_.append(
                            (lambda s, b, i2: lambda: q_sub_tail(s, nrms, i2, nqros, b[0]))(sc, pq_box, i)
                        )

                for jc in range(16):
                    pT_t = ptp.tile([128, 4, 512], BF16, tag="pT")
                    pts.append(pT_t)
                    if use_mask:
                        mkt = mp.tile([128, 512], F32, tag="mkt")
                        nc.sync.dma_start(out=mkt[:], in_=mk[jc][:, isl])
                    for pair in range(2):
                        pss = ps.tile([128, 2, 512], F32,
                                      tag=("scA" if pair == 0 else "scB"))
                        for hh in range(2):
                            rows = slice(64 * hh, 64 * hh + 64)
                            nc.tensor.matmul(
                                pss[:, hh, :],
                                lhsT=kT[rows, jc * 128:(jc + 1) * 128],
                                rhs=qT[rows, pair, isl],
                                start=True, stop=True,
                            )
                        if use_mask:
                            sm = mp.tile([128, 2, 512], F32, tag="sm")
                            nc.vector.scalar_tensor_tensor(
                                sm[:], pss[:], 0.125,
                                mkt[:, None, :].to_broadcast((128, 2, 512)),
                                AL.mult, AL.add,
                            )
                            nc.scalar.activation(
                                pT_t[:, 2 * pair:2 * pair + 2, :], sm[:], AF.Exp
                            )
                        else:
                            nc.scalar.activation(
                                pT_t[:, 2 * pair:2 * pair + 2, :], pss[:], AF.Exp,
                                scale=0.125,
                            )
                    # this block's first-half PV, lagged behind the spine;
                    # the last block also tracks its second half live in the
                    # pq/py psum slots (idle there: no further q proj)
                    wpv.append(pv_group(state[ic]["o0"], pts, jc, (0, 1)))
                    if ic == 3:
                        wpv.append(pv_group1(state[3]["o1"], "pq", pts, jc, 2))
                        wpv.append(pv_group1(state[3]["o1b"], "py", pts, jc, 3))
                    pump(wpv, 4 if ic < 3 else 6)
                    if ic < 3 and jc == 9:
                        pump(wq_, len(wq_))
                        q_fin(ic + 1, nrms, nqros)
                    else:
                        pump(wq_, 2)

            # tail: finish last block (both halves)
            pv = state[3]
            pump(wpv, len(wpv))
            norm_half(0, pv["o0"][0], pv["oT0"])
            outproj_sub(12, pv["oT0"][0])
            outproj_sub(13, pv["oT0"][1])
            norm_sub(pv["o1"][0], pv["oT1"])
            norm_sub(pv["o1b"][0], pv["oT1"])
            outproj_sub(14, pv["oT1"][0])
            outproj_sub(15, pv["oT1"][1])

    nc.compile()
    return nc


def _get(use_mask: bool, use_bias: bool = False):
    key = (use_mask, use_bias)
    if key not in _cache:
        _cache[key] = _build(use_mask, use_bias)
    return _cache[key]


def _host_prep(hidden_state, attention_mask, Wq, bq, Wk, bk, Wv, bv, Wo,
               use_mask, use_bias):
    """Build the 8 per-core input maps."""
    half_q, half_k = HID // 2, (HKV * D) // 2  # 512, 128
    inv_q = ROPE_BASE ** (-np.arange(half_q, dtype=np.float64) / half_q)
    inv_k = ROPE_BASE ** (-np.arange(half_k, dtype=np.float64) / half_k)
    s_idx = np.arange(S, dtype=np.float64)
    ang_q = inv_q[:, None] * s_idx[None, :]  # [512, S] freq-major
    ang_k = inv_k[:, None] * s_idx[None, :]  # [128, S]
    cos_q, sin_q = np.cos(ang_q), np.sin(ang_q)
    cos_k, sin_k = np.cos(ang_k), np.sin(ang_k)

    in_maps = []
    for core in range(8):
        b, j = core // 4, core % 4
        own_q = np.arange(j * 256, (j + 1) * 256)
        par_q = own_q + 512 if j < 2 else own_q - 512
        fidx_q = own_q if j < 2 else own_q - 512
        sign = -1.0 if j < 2 else 1.0
        own_k = np.arange(j * 64, (j + 1) * 64)
        par_k = own_k + 128 if j < 2 else own_k - 128
        fidx_k = own_k if j < 2 else own_k - 128

        hTc = np.ascontiguousarray(hidden_state[b].T).astype(NB).reshape(8, 128, S)
        wq_c = np.concatenate([Wq[:, own_q], Wq[:, par_q]], axis=1)
        wq_c = wq_c.astype(NB).reshape(8, 128, QSEL)
        wk_c = np.concatenate([Wk[:, own_k], Wk[:, par_k]], axis=1)
        wk_c = wk_c.astype(NB).reshape(8, 128, 128)
        wv_c = Wv[:, own_k].astype(NB).reshape(8, 128, 64)
        wo_c = Wo[j * 256:(j + 1) * 256, :].astype(NB).reshape(2, 128, HID)
        # natural-layout cos/sin: [sc, seq128, {cos, signed sin}, feat]
        csq_c = np.stack(
            [cos_q[fidx_q].T, (sign * sin_q[fidx_q]).T], axis=1
        )  # [S, 2, 256]
        csq_c = csq_c.astype(NB).reshape(NSC, 128, 2, 256)
        csk_c = np.stack(
            [cos_k[fidx_k].T, (sign * sin_k[fidx_k]).T], axis=1
        )  # [S, 2, 64]
        csk_c = csk_c.astype(NB).reshape(NSC, 128, 2, 64)

        m = {
            "hT": hTc, "wq": wq_c, "wk": wk_c, "wv": wv_c, "wo": wo_c,
            "csq": csq_c, "csk": csk_c,
        }
        if use_mask:
            mT = np.ascontiguousarray(attention_mask[b].T).astype(np.float32)
            m["mk"] = mT.reshape(NSC, 128, S)
        if use_bias:
            m["brq"] = np.concatenate([bq[own_q], bq[par_q]]).astype(NB).reshape(1, QSEL)
            m["brk"] = np.concatenate([bk[own_k], bk[par_k]]).astype(NB).reshape(1, 128)
            m["brv"] = bv[own_k].astype(NB).reshape(1, 64)
        in_maps.append(m)
    return in_maps


def kernel(hidden_state, attention_mask, Wq, bq, Wk, bk, Wv, bv, Wo, bo):
    from concourse.bass_utils import run_bass_kernel_spmd

    hidden_state = np.asarray(hidden_state, dtype=np.float32)
    attention_mask = np.asarray(attention_mask, dtype=np.float32)
    Wq, bq = np.asarray(Wq, np.float32), np.asarray(bq, np.float32)
    Wk, bk = np.asarray(Wk, np.float32), np.asarray(bk, np.float32)
    Wv, bv = np.asarray(Wv, np.float32), np.asarray(bv, np.float32)
    Wo, bo = np.asarray(Wo, np.float32), np.asarray(bo, np.float32)
    use_mask = bool(np.any(attention_mask))
    use_bias = bool(np.any(bq) or np.any(bk) or np.any(bv))
    nc = _get(use_mask, use_bias)
    in_maps = _host_prep(
        hidden_state, attention_mask, Wq, bq, Wk, bk, Wv, bv, Wo,
        use_mask, use_bias,
    )
    res = run_bass_kernel_spmd(nc, in_maps, list(range(8)))
    out = np.zeros((B, S, HID), dtype=np.float32)
    for core in range(8):
        out[core // 4] += res.results[core]["y"].reshape(S, HID)
    out += bo[None, None, :]
    return out
